# revision 4
# baseline (speedup 1.0000x reference)
"""Trainium2 Bass kernel for nn_CPModule_9019431321787 (retrieval_knn).

Contract: kernel(**inputs) takes the FULL unsharded inputs
(x [2,4,64,32,32] f32 + MLP weights) and returns the FULL output
[2,4,64,32,32] f32, running SPMD on 8 NeuronCores.

Algorithm notes (all derived offline, hardcoded here):
  - The 3-layer MLP has no activations, so it folds into ONE linear map
    Wc [131,64], bc [64].  Split Wc rows: Wq (query feats), Wn (neighbor
    feats), Wd (displacement).
  - out[b,i,:] = max_k( YP[b, idx_k, :] ) + A[b,i,:], where
        YP[j] = x_j . Wn + pos_j . Wd          (candidate table)
        A[i]  = x_i . Wq + bc + qpos_i . Wd    (per-query, k-invariant)
    because the query-side terms are constant across k and max is
    translation-equivariant.
  - top-k selection: top_k(sim) == top-8 of z = 2*q.c - |c|^2 per query
    row (monotone transform of the reference's -sqrt(clip(d2)) in the
    realized value range; the k-max at the end is order-invariant).
  - same-frame masking: queries on a core are exactly one time frame
    (1024 rows); the masked candidates are dropped host-side, so each
    core only scores its 3072 allowed candidates (local indices).
  - Sharding: core c handles batch c//4, query frame c%4.  Fully data
    parallel, no collectives.
"""

import numpy as np

BS, T, FEAT, H, W = 2, 4, 64, 32, 32
HWP = H * W            # 1024
THW = T * HWP          # 4096
K = 8
NCORES = 8
CAND = (T - 1) * HWP   # 3072 allowed candidates per core
QTILES = HWP // 128    # 8 query tiles of 128 rows
CTILES = CAND // 128   # 24 candidate tiles

_COMPILED = {}


def _build_nc():
    import concourse.bacc as bacc
    import concourse.bass as bass
    import concourse.mybir as mybir
    import concourse.tile as tile

    f32 = mybir.dt.float32

    nc = bacc.Bacc(
        "TRN2",
        target_bir_lowering=False,
        debug=False,
        enable_asserts=False,
        num_devices=NCORES,
    )

    qT_d = nc.dram_tensor("qT", [FEAT, HWP], f32, kind="ExternalInput")
    cT_d = nc.dram_tensor("cT", [FEAT, CAND], f32, kind="ExternalInput")
    wq_d = nc.dram_tensor("Wq", [FEAT, FEAT], f32, kind="ExternalInput")
    wn_d = nc.dram_tensor("Wn", [FEAT, FEAT], f32, kind="ExternalInput")
    id_d = nc.dram_tensor("I128", [128, 128], f32, kind="ExternalInput")
    at_d = nc.dram_tensor("Atab", [128, QTILES * FEAT], f32, kind="ExternalInput")
    pt_d = nc.dram_tensor("Ptab", [128, CTILES * FEAT], f32, kind="ExternalInput")
    out_d = nc.dram_tensor("out", [HWP, FEAT], f32, kind="ExternalOutput")

    with tile.TileContext(nc) as tc:
        with (
            tc.tile_pool(name="const", bufs=1) as cpool,
            tc.tile_pool(name="zpsum", bufs=2, space="PSUM") as zp_pool,
            tc.tile_pool(name="spsum", bufs=2, space="PSUM") as sp_pool,
            tc.tile_pool(name="zsb", bufs=2) as zsb_pool,
            tc.tile_pool(name="small", bufs=3) as small_pool,
            tc.tile_pool(name="dram", bufs=1, space="DRAM") as dram_pool,
            tc.tile_pool(name="dram2", bufs=2, space="DRAM") as dram2_pool,
        ):
            # ---- constant loads ----
            ct = cpool.tile([FEAT + 1, CAND], f32)   # row 64 will hold -|c|^2
            nc.sync.dma_start(out=ct[0:FEAT, :], in_=cT_d.ap())
            qt = cpool.tile([FEAT, HWP], f32)
            nc.sync.dma_start(out=qt[:], in_=qT_d.ap())
            wq = cpool.tile([FEAT, FEAT], f32)
            nc.sync.dma_start(out=wq[:], in_=wq_d.ap())
            wn = cpool.tile([FEAT, FEAT], f32)
            nc.sync.dma_start(out=wn[:], in_=wn_d.ap())
            ident = cpool.tile([128, 128], f32)
            nc.sync.dma_start(out=ident[:], in_=id_d.ap())
            atab = cpool.tile([128, QTILES * FEAT], f32)
            nc.sync.dma_start(out=atab[:], in_=at_d.ap())
            ptab = cpool.tile([128, CTILES * FEAT], f32)
            nc.sync.dma_start(out=ptab[:], in_=pt_d.ap())
            negones = cpool.tile([FEAT, 1], f32)
            nc.vector.memset(negones[:], -1.0)

            # ---- -|c|^2 into ct row 64 (via squares + ones-matmul) ----
            sqt = cpool.tile([FEAT, CAND], f32)
            nc.scalar.square(sqt[:], ct[0:FEAT, :])
            for h in range(2):
                zp = zp_pool.tile([128, 1536], f32, tag="z")
                for s in range(3):
                    sl = slice(s * 512, (s + 1) * 512)
                    nc.tensor.matmul(
                        out=zp[FEAT : FEAT + 1, sl],
                        lhsT=negones[:],
                        rhs=sqt[:, h * 1536 + s * 512 : h * 1536 + (s + 1) * 512],
                        start=True,
                        stop=True,
                    )
                nc.scalar.copy(
                    out=ct[FEAT : FEAT + 1, h * 1536 : (h + 1) * 1536],
                    in_=zp[FEAT : FEAT + 1, :],
                )

            # ---- candidate table YP = c.Wn + Ptab  -> DRAM ----
            ypd = dram_pool.tile([CAND, FEAT], f32)
            for r in range(CTILES):
                yp = sp_pool.tile([128, FEAT], f32, tag="sp")
                nc.tensor.matmul(
                    out=yp[:],
                    lhsT=ct[0:FEAT, r * 128 : (r + 1) * 128],
                    rhs=wn[:],
                    start=True,
                    stop=False,
                )
                nc.tensor.matmul(
                    out=yp[:],
                    lhsT=ident[:],
                    rhs=ptab[:, r * FEAT : (r + 1) * FEAT],
                    start=False,
                    stop=True,
                )
                yp_sb = small_pool.tile([128, FEAT], f32, tag="ypsb")
                nc.scalar.copy(out=yp_sb[:], in_=yp[:])
                nc.sync.dma_start(out=ypd[r * 128 : (r + 1) * 128, :], in_=yp_sb[:])

            # ---- per query tile: z matmul -> top8 -> gather -> kmax + A ----
            for q in range(QTILES):
                qsl = slice(q * 128, (q + 1) * 128)
                aug = small_pool.tile([FEAT + 1, 128], f32, tag="aug")
                nc.scalar.mul(aug[0:FEAT, :], qt[:, qsl], 2.0)
                nc.gpsimd.memset(aug[FEAT : FEAT + 1, :], 1.0)

                zsb = zsb_pool.tile([128, CAND], f32, tag="zsb")
                for h in range(2):
                    zp = zp_pool.tile([128, 1536], f32, tag="z")
                    for s in range(3):
                        nc.tensor.matmul(
                            out=zp[:, s * 512 : (s + 1) * 512],
                            lhsT=aug[:],
                            rhs=ct[:, h * 1536 + s * 512 : h * 1536 + (s + 1) * 512],
                            start=True,
                            stop=True,
                        )
                    nc.scalar.copy(
                        out=zsb[:, h * 1536 : (h + 1) * 1536], in_=zp[:]
                    )

                vals = small_pool.tile([128, K], f32, tag="vals")
                idx = small_pool.tile([128, K], mybir.dt.uint16, tag="idx")
                nc.vector.max(out=vals[:], in_=zsb[:])
                nc.vector.max_index(out=idx[:], in_max=vals[:], in_values=zsb[:])

                # Build the dma_gather index layout: flat slot i = k*128+p must
                # hold idx[p, k], stored at idxs partition i%16, column i//16,
                # replicated across the 8 gpsimd-core stripes.
                d3 = dram2_pool.tile([128, K], mybir.dt.int16, tag="d3")
                nc.sync.dma_start(out=d3[:], in_=idx[:].bitcast(mybir.dt.int16))
                idxs_g = small_pool.tile([128, 64], mybir.dt.int16, tag="idxs_g")
                for k in range(K):
                    src = d3[:, k : k + 1].rearrange("(phi c) one -> c phi one", c=16)
                    nc.sync.dma_start(out=idxs_g[0:16, k * 8 : (k + 1) * 8], in_=src)
                nc.sync.dma_start(out=idxs_g[16:32, :], in_=idxs_g[0:16, :])
                nc.sync.dma_start(out=idxs_g[32:64, :], in_=idxs_g[0:32, :])
                nc.sync.dma_start(out=idxs_g[64:128, :], in_=idxs_g[0:64, :])

                g = small_pool.tile([128, K, FEAT], f32, tag="g")
                nc.gpsimd.dma_gather(
                    out_ap=g[:],
                    in_ap=ypd[:],
                    idxs_ap=idxs_g[:],
                    num_idxs=128 * K,
                    num_idxs_reg=128 * K,
                    elem_size=FEAT,
                )

                apsum = sp_pool.tile([128, FEAT], f32, tag="sp")
                nc.tensor.matmul(
                    out=apsum[:], lhsT=qt[:, qsl], rhs=wq[:], start=True, stop=False
                )
                nc.tensor.matmul(
                    out=apsum[:],
                    lhsT=ident[:],
                    rhs=atab[:, q * FEAT : (q + 1) * FEAT],
                    start=False,
                    stop=True,
                )

                gmax = small_pool.tile([128, FEAT], f32, tag="gmax")
                nc.vector.tensor_reduce(
                    out=gmax[:],
                    in_=g[:].rearrange("p k f -> p f k"),
                    op=mybir.AluOpType.max,
                    axis=mybir.AxisListType.X,
                )
                outsb = small_pool.tile([128, FEAT], f32, tag="outsb")
                nc.vector.tensor_add(out=outsb[:], in0=gmax[:], in1=apsum[:])
                nc.sync.dma_start(out=out_d.ap()[qsl, :], in_=outsb[:])

    nc.compile()
    return nc


def _prep_in_maps(inputs):
    x = np.ascontiguousarray(np.asarray(inputs["x"], np.float32))
    W1 = np.asarray(inputs["W1"], np.float64)
    b1 = np.asarray(inputs["b1"], np.float64)
    W2 = np.asarray(inputs["W2"], np.float64)
    b2 = np.asarray(inputs["b2"], np.float64)
    W3 = np.asarray(inputs["W3"], np.float64)
    b3 = np.asarray(inputs["b3"], np.float64)

    Wc = W1.T @ W2.T @ W3.T                      # [131, 64]
    bc = b1 @ W2.T @ W3.T + b2 @ W3.T + b3       # [64]
    Wq = np.ascontiguousarray(Wc[:FEAT]).astype(np.float32)
    Wn = np.ascontiguousarray(Wc[FEAT : 2 * FEAT]).astype(np.float32)
    Wd = Wc[2 * FEAT :]                          # [3, 64] keep f64 for tables

    I128 = np.eye(128, dtype=np.float32)

    in_maps = []
    for c in range(NCORES):
        b, f = c // 4, c % 4
        frames = [t for t in range(T) if t != f]
        qT = x[b, f].reshape(FEAT, HWP)
        cT = np.concatenate([x[b, t].reshape(FEAT, HWP) for t in frames], axis=1)

        jglob = np.concatenate(
            [np.arange(t * HWP, (t + 1) * HWP) for t in frames]
        )
        ctp = (jglob // HWP).astype(np.float64) / T
        chp = ((jglob % HWP) // W).astype(np.float64)
        cwp = ((jglob % HWP) % W).astype(np.float64)
        Ptab = (np.stack([ctp, chp, cwp], -1) @ Wd).astype(np.float32)  # [3072,64]

        iq = np.arange(f * HWP, (f + 1) * HWP)
        it = ((iq // H) * W).astype(np.float64) / T
        ih = (((iq % H) * W) // W).astype(np.float64)
        iw = (((iq % H) * W) % W).astype(np.float64)
        Atab = (bc + np.stack([it, ih, iw], -1) @ Wd).astype(np.float32)  # [1024,64]

        # device layout: partition p <-> row (tile*128 + p)
        Atab_l = np.ascontiguousarray(
            Atab.reshape(QTILES, 128, FEAT).transpose(1, 0, 2).reshape(128, -1)
        )
        Ptab_l = np.ascontiguousarray(
            Ptab.reshape(CTILES, 128, FEAT).transpose(1, 0, 2).reshape(128, -1)
        )

        in_maps.append(
            {
                "qT": np.ascontiguousarray(qT),
                "cT": np.ascontiguousarray(cT),
                "Wq": Wq,
                "Wn": Wn,
                "I128": I128,
                "Atab": Atab_l,
                "Ptab": Ptab_l,
            }
        )
    return in_maps


def run_with_results(inputs, trace=False, **spmd_kwargs):
    """Run the SPMD kernel; returns (full_output, BassKernelResults)."""
    from concourse import bass_utils

    if "nc" not in _COMPILED:
        _COMPILED["nc"] = _build_nc()
    nc = _COMPILED["nc"]

    in_maps = _prep_in_maps(inputs)
    res = bass_utils.run_bass_kernel_spmd(
        nc, in_maps, core_ids=list(range(NCORES)), trace=trace, **spmd_kwargs
    )

    y = np.zeros((BS, THW, FEAT), np.float32)
    for c in range(NCORES):
        b, f = c // 4, c % 4
        y[b, f * HWP : (f + 1) * HWP] = res.results[c]["out"]
    out = y.reshape(BS, T, H, W, FEAT).transpose(0, 1, 4, 2, 3)
    return np.ascontiguousarray(out), res


def kernel(**inputs):
    out, _ = run_with_results(inputs, trace=False)
    return out


# revision 11
# speedup vs baseline: 1.2022x; 1.2022x over previous
"""Trainium2 Bass kernel for nn_CPModule_9019431321787 (retrieval_knn).

kernel(**inputs) takes the FULL unsharded inputs (x [2,4,64,32,32] f32 +
MLP weights) and returns the FULL output [2,4,64,32,32] f32, running
SPMD on 8 NeuronCores (core c = batch c//4, query time-frame c%4; fully
data-parallel, no collectives).

Math (derived offline):
  - The activation-free MLP folds to one linear map Wc [131,64], bc.
  - out[b,i,:] = max_k YP[idx_k,:] + A[i,:], with
      YP[j] = c_j.Wn + pos_j.Wd   (candidate table, gathered by top-k)
      A[i]  = q_i.Wq + bc + qpos_i.Wd   (k-invariant, pulled out of max)
  - top-8 by z = 2 q.c - |c|^2 (monotone to the reference similarity);
    same-frame candidates are excluded host-side (3072 left per core).
  - z is computed as ONE K=68 matmul per PSUM bank: rows = [2q | 0 0 0 | 1]
    against candidate matrix [c | pos^T | -|c|^2], so the pos rows feed the
    YP matmul (K=67) and the -|c|^2 row feeds z, with no extra adds.
  - All matmuls run fp32r (HW reduced-precision fp32, ~13-bit mantissa,
    abs err ~6e-3 on K=65 dots) - far inside the output tolerance, and it
    flips only O(100/8192) boundary top-k rows (near-equidistant ties).
  - top-8 values+indices via the DVE MAX8 / FIND_INDEX8 instructions;
    neighbor rows fetched with the custom SWDGE dma_gather (idx shuffled
    into its 16-partition wrap layout via small DMAs through DRAM).
"""

import numpy as np

BS, T, FEAT, H, W = 2, 4, 64, 32, 32
HWP = H * W            # 1024
THW = T * HWP          # 4096
K = 8
NCORES = 8
CAND = (T - 1) * HWP   # 3072 allowed candidates per core
QTILES = HWP // 128    # 8 query tiles of 128 rows
CTILES = CAND // 128   # 24 candidate tiles
KAUG = FEAT + 4        # 68 = feats + 3 pos rows + (-|c|^2) row

_COMPILED = {}


def _build_nc():
    import concourse.bacc as bacc
    import concourse.mybir as mybir
    import concourse.tile as tile

    f32 = mybir.dt.float32
    f32r = mybir.dt.float32r
    i16 = mybir.dt.int16

    nc = bacc.Bacc(
        "TRN2",
        target_bir_lowering=False,
        debug=False,
        enable_asserts=False,
        num_devices=NCORES,
        num_swdge_queues=4,
    )

    qT_d = nc.dram_tensor("qT", [KAUG, HWP], f32, kind="ExternalInput")
    cT_d = nc.dram_tensor("cT", [FEAT, CAND], f32r, kind="ExternalInput")
    posT_d = nc.dram_tensor("posT", [3, CAND], f32r, kind="ExternalInput")
    wq_d = nc.dram_tensor("Wq2", [FEAT, FEAT], f32r, kind="ExternalInput")
    wnd_d = nc.dram_tensor("Wnd", [FEAT + 3, FEAT], f32r, kind="ExternalInput")
    id_d = nc.dram_tensor("I128", [128, 128], f32r, kind="ExternalInput")
    at_d = nc.dram_tensor("Atab", [128, QTILES * FEAT], f32r, kind="ExternalInput")
    neg_d = nc.dram_tensor("negones", [FEAT, FEAT], f32, kind="ExternalInput")
    out_d = nc.dram_tensor("out", [HWP, FEAT], f32, kind="ExternalOutput")

    with tile.TileContext(nc) as tc:
        with (
            tc.tile_pool(name="const", bufs=1) as cpool,
            tc.tile_pool(name="zpsum", bufs=2, space="PSUM") as zp_pool,
            tc.tile_pool(name="apsum", bufs=1, space="PSUM") as ap_pool,
            tc.tile_pool(name="ypsum", bufs=1, space="PSUM") as yp_pool,
            tc.tile_pool(name="zsb", bufs=2) as zsb_pool,
            tc.tile_pool(name="small", bufs=3) as small_pool,
            tc.tile_pool(name="dram", bufs=1, space="DRAM") as dram_pool,
            tc.tile_pool(name="dram2", bufs=2, space="DRAM") as dram2_pool,
        ):
            # ---- constant loads ----
            ct = cpool.tile([KAUG, CAND], f32r)  # [c | pos^T | -|c|^2]
            nc.sync.dma_start(out=ct[0:FEAT, :], in_=cT_d.ap())
            qt_f = cpool.tile([KAUG, HWP], f32)
            nc.sync.dma_start(out=qt_f[:], in_=qT_d.ap())
            wq = cpool.tile([FEAT, FEAT], f32r)
            nc.sync.dma_start(out=wq[:], in_=wq_d.ap())
            wnd = cpool.tile([FEAT + 3, FEAT], f32r)
            nc.sync.dma_start(out=wnd[:], in_=wnd_d.ap())
            ident = cpool.tile([128, 128], f32r)
            nc.sync.dma_start(out=ident[:], in_=id_d.ap())
            atab = cpool.tile([128, QTILES * FEAT], f32r)
            nc.sync.dma_start(out=atab[:], in_=at_d.ap())
            # 64 columns of -1 (host input): the |c|^2 matmul writes 64
            # identical rows at PSUM base partition 64 (fp32 - fp32r cannot
            # target base 64) so row 67 can be copied same-partition into ct.
            negones = cpool.tile([FEAT, FEAT], f32)
            nc.sync.dma_start(out=negones[:], in_=neg_d.ap())

            # ---- query matrix [2q | 0 0 0 | 1] (host pads rows 64:67=0,
            # row 67=0.5; doubling gives the 1) ----
            qt = cpool.tile([KAUG, HWP], f32r)
            nc.scalar.mul(qt[:], qt_f[:], 2.0)

            # ---- -|c|^2 into ct row 67 ----
            sqt = cpool.tile([FEAT, CAND], f32r)
            nc.scalar.square(sqt[:], ct[0:FEAT, :])
            for h in range(2):
                zp = zp_pool.tile([128, 1536], f32, tag="z")
                for s in range(3):
                    nc.tensor.matmul(
                        out=zp[FEAT : 128, s * 512 : (s + 1) * 512],
                        lhsT=negones[:],
                        rhs=sqt[:, h * 1536 + s * 512 : h * 1536 + (s + 1) * 512].bitcast(
                            f32
                        ),
                        start=True,
                        stop=True,
                    )
                # copy all 4 replicated rows (engine start-partition must be
                # 64); rows 64:66 are then overwritten by the posT DMA below.
                nc.scalar.copy(
                    out=ct[FEAT : FEAT + 4, h * 1536 : (h + 1) * 1536],
                    in_=zp[FEAT : FEAT + 4, :],
                )

            # posT overwrites rows 64:67 AFTER the -|c|^2 copy (WAW dep)
            nc.sync.dma_start(out=ct[FEAT : FEAT + 3, :], in_=posT_d.ap())

            # ---- candidate table YP = [c|pos].Wnd -> DRAM (4-chunk groups) --
            ypd = dram_pool.tile([CAND, FEAT], f32)
            for grp in range(CTILES // 4):
                yp4 = yp_pool.tile([128, 4 * FEAT], f32, tag="yp")
                for j in range(4):
                    r = grp * 4 + j
                    nc.tensor.matmul(
                        out=yp4[:, j * FEAT : (j + 1) * FEAT],
                        lhsT=ct[0 : FEAT + 3, r * 128 : (r + 1) * 128],
                        rhs=wnd[:],
                        start=True,
                        stop=True,
                    )
                yp_sb = small_pool.tile([128, 4 * FEAT], f32, tag="ypsb")
                nc.scalar.copy(out=yp_sb[:], in_=yp4[:])
                dst = ypd[grp * 512 : (grp + 1) * 512, :].rearrange(
                    "(g p) f -> p g f", p=128
                )
                nc.sync.dma_start(out=dst, in_=yp_sb[:].rearrange("p (g f) -> p g f", g=4))

            # ---- A bank: A = 2q.(Wq/2) + Atab, one PSUM bank, all tiles ----
            abank = ap_pool.tile([128, QTILES * FEAT], f32, tag="a")
            for q in range(QTILES):
                csl = slice(q * FEAT, (q + 1) * FEAT)
                nc.tensor.matmul(
                    out=abank[:, csl],
                    lhsT=qt[0:FEAT, q * 128 : (q + 1) * 128],
                    rhs=wq[:],
                    start=True,
                    stop=False,
                )
                nc.tensor.matmul(
                    out=abank[:, csl],
                    lhsT=ident[:],
                    rhs=atab[:, csl],
                    start=False,
                    stop=True,
                )

            # ---- per query tile ----
            for q in range(QTILES):
                qsl = slice(q * 128, (q + 1) * 128)
                zsb = zsb_pool.tile([128, CAND], f32, tag="zsb")
                for h in range(2):
                    zp = zp_pool.tile([128, 1536], f32, tag="z")
                    for s in range(3):
                        nc.tensor.matmul(
                            out=zp[:, s * 512 : (s + 1) * 512],
                            lhsT=qt[:, qsl],
                            rhs=ct[:, h * 1536 + s * 512 : h * 1536 + (s + 1) * 512],
                            start=True,
                            stop=True,
                        )
                    nc.scalar.copy(out=zsb[:, h * 1536 : (h + 1) * 1536], in_=zp[:])

                vals = small_pool.tile([128, K], f32, tag="vals")
                idx = small_pool.tile([128, K], mybir.dt.uint16, tag="idx")
                nc.vector.max(out=vals[:], in_=zsb[:])
                nc.vector.max_index(out=idx[:], in_max=vals[:], in_values=zsb[:])

                # shuffle into dma_gather's wrap layout (via DRAM bounce):
                # idxs_g[c, k*8+phi] = idx[phi*16+c, k], replicated per stripe
                d3 = dram2_pool.tile([128, K], i16, tag="d3")
                nc.sync.dma_start(out=d3[:], in_=idx[:].bitcast(i16))
                idxs_g = small_pool.tile([128, 64], i16, tag="idxs_g")
                for k in range(K):
                    src = d3[:, k : k + 1].rearrange("(phi c) one -> c phi one", c=16)
                    nc.sync.dma_start(out=idxs_g[0:16, k * 8 : (k + 1) * 8], in_=src)
                nc.sync.dma_start(out=idxs_g[16:32, :], in_=idxs_g[0:16, :])
                nc.sync.dma_start(out=idxs_g[32:64, :], in_=idxs_g[0:32, :])
                nc.sync.dma_start(out=idxs_g[64:128, :], in_=idxs_g[0:64, :])

                g = small_pool.tile([128, K, FEAT], f32, tag="g")
                nc.gpsimd.dma_gather(
                    out_ap=g[:],
                    in_ap=ypd[:],
                    idxs_ap=idxs_g[:],
                    num_idxs=128 * K,
                    num_idxs_reg=128 * K,
                    elem_size=FEAT,
                    queue_num=q % 4,
                )

                gmax = small_pool.tile([128, FEAT], f32, tag="gmax")
                nc.vector.tensor_reduce(
                    out=gmax[:],
                    in_=g[:].rearrange("p k f -> p f k"),
                    op=mybir.AluOpType.max,
                    axis=mybir.AxisListType.X,
                )
                outsb = small_pool.tile([128, FEAT], f32, tag="outsb")
                nc.vector.tensor_add(
                    out=outsb[:], in0=gmax[:], in1=abank[:, q * FEAT : (q + 1) * FEAT]
                )
                nc.sync.dma_start(out=out_d.ap()[qsl, :], in_=outsb[:])

    nc.compile()
    return nc


def _prep_in_maps(inputs):
    x = np.ascontiguousarray(np.asarray(inputs["x"], np.float32))
    W1 = np.asarray(inputs["W1"], np.float64)
    b1 = np.asarray(inputs["b1"], np.float64)
    W2 = np.asarray(inputs["W2"], np.float64)
    b2 = np.asarray(inputs["b2"], np.float64)
    W3 = np.asarray(inputs["W3"], np.float64)
    b3 = np.asarray(inputs["b3"], np.float64)

    Wc = W1.T @ W2.T @ W3.T                      # [131, 64]
    bc = b1 @ W2.T @ W3.T + b2 @ W3.T + b3       # [64]
    Wq2 = np.ascontiguousarray(Wc[:FEAT] / 2.0).astype(np.float32)
    Wn = Wc[FEAT : 2 * FEAT]
    Wd = Wc[2 * FEAT :]                          # [3, 64]
    Wnd = np.ascontiguousarray(np.vstack([Wn, Wd])).astype(np.float32)

    I128 = np.eye(128, dtype=np.float32)

    in_maps = []
    for c in range(NCORES):
        b, f = c // 4, c % 4
        frames = [t for t in range(T) if t != f]
        qT = np.zeros((KAUG, HWP), np.float32)
        qT[0:FEAT] = x[b, f].reshape(FEAT, HWP)
        qT[FEAT + 3] = 0.5
        cT = np.concatenate([x[b, t].reshape(FEAT, HWP) for t in frames], axis=1)

        jglob = np.concatenate(
            [np.arange(t * HWP, (t + 1) * HWP) for t in frames]
        )
        ctp = (jglob // HWP).astype(np.float64) / T
        chp = ((jglob % HWP) // W).astype(np.float64)
        cwp = ((jglob % HWP) % W).astype(np.float64)
        posT = np.ascontiguousarray(
            np.stack([ctp, chp, cwp], 0).astype(np.float32)
        )  # [3, 3072]

        iq = np.arange(f * HWP, (f + 1) * HWP)
        it = ((iq // H) * W).astype(np.float64) / T
        ih = (((iq % H) * W) // W).astype(np.float64)
        iw = (((iq % H) * W) % W).astype(np.float64)
        Atab = (bc + np.stack([it, ih, iw], -1) @ Wd).astype(np.float32)  # [1024,64]
        Atab_l = np.ascontiguousarray(
            Atab.reshape(QTILES, 128, FEAT).transpose(1, 0, 2).reshape(128, -1)
        )

        in_maps.append(
            {
                "qT": np.ascontiguousarray(qT),
                "cT": np.ascontiguousarray(cT),
                "posT": posT,
                "Wq2": Wq2,
                "Wnd": Wnd,
                "I128": I128,
                "negones": np.full((FEAT, FEAT), -1.0, np.float32),
                "Atab": Atab_l,
            }
        )
    return in_maps


def run_with_results(inputs, trace=False, **spmd_kwargs):
    """Run the SPMD kernel; returns (full_output, BassKernelResults)."""
    from concourse import bass_utils

    if "nc" not in _COMPILED:
        _COMPILED["nc"] = _build_nc()
    nc = _COMPILED["nc"]

    in_maps = _prep_in_maps(inputs)
    res = bass_utils.run_bass_kernel_spmd(
        nc, in_maps, core_ids=list(range(NCORES)), trace=trace, **spmd_kwargs
    )

    y = np.zeros((BS, THW, FEAT), np.float32)
    for c in range(NCORES):
        b, f = c // 4, c % 4
        y[b, f * HWP : (f + 1) * HWP] = res.results[c]["out"]
    out = y.reshape(BS, T, H, W, FEAT).transpose(0, 1, 4, 2, 3)
    return np.ascontiguousarray(out), res


def kernel(**inputs):
    out, _ = run_with_results(inputs, trace=False)
    return out


# revision 13
# speedup vs baseline: 1.4158x; 1.1777x over previous
"""Trainium2 Bass kernel for nn_CPModule_9019431321787 (retrieval_knn).

kernel(**inputs) takes the FULL unsharded inputs (x [2,4,64,32,32] f32 +
MLP weights) and returns the FULL output [2,4,64,32,32] f32, running
SPMD on 8 NeuronCores (core c = batch c//4, query time-frame c%4; fully
data-parallel, no collectives).

Math (derived offline):
  - The activation-free MLP folds to one linear map Wc [131,64], bc.
  - out[b,i,:] = max_k YP[idx_k,:] + A[i,:], with
      YP[j] = c_j.Wn + pos_j.Wd   (candidate table, gathered by top-k)
      A[i]  = q_i.Wq + bc + qpos_i.Wd   (k-invariant, pulled out of max)
  - top-8 by z = 2 q.c - |c|^2 (monotone to the reference similarity);
    same-frame candidates are excluded host-side (3072 left per core).
  - z is computed as ONE K=68 matmul per PSUM bank: rows = [2q | 0 0 0 | 1]
    against candidate matrix [c | pos^T | -|c|^2], so the pos rows feed the
    YP matmul (K=67) and the -|c|^2 row feeds z, with no extra adds.
  - All matmuls run fp32r (HW reduced-precision fp32, ~13-bit mantissa,
    abs err ~6e-3 on K=65 dots) - far inside the output tolerance, and it
    flips only O(100/8192) boundary top-k rows (near-equidistant ties).
  - top-8 values+indices via the DVE MAX8 / FIND_INDEX8 instructions;
    neighbor rows fetched with the custom SWDGE dma_gather (idx shuffled
    into its 16-partition wrap layout via small DMAs through DRAM).
"""

import numpy as np

BS, T, FEAT, H, W = 2, 4, 64, 32, 32
HWP = H * W            # 1024
THW = T * HWP          # 4096
K = 8
NCORES = 8
CAND = (T - 1) * HWP   # 3072 allowed candidates per core
QTILES = HWP // 128    # 8 query tiles of 128 rows
CTILES = CAND // 128   # 24 candidate tiles
KAUG = FEAT + 4        # 68 = feats + 3 pos rows + (-|c|^2) row

_COMPILED = {}


def _build_nc():
    import concourse.bacc as bacc
    import concourse.mybir as mybir
    import concourse.tile as tile

    f32 = mybir.dt.float32
    f32r = mybir.dt.float32r
    i16 = mybir.dt.int16

    nc = bacc.Bacc(
        "TRN2",
        target_bir_lowering=False,
        debug=False,
        enable_asserts=False,
        num_devices=NCORES,
        num_swdge_queues=4,
    )

    qT_d = nc.dram_tensor("qT", [KAUG, HWP], f32, kind="ExternalInput")
    cT_d = nc.dram_tensor("cT", [FEAT, CAND], f32r, kind="ExternalInput")
    posT_d = nc.dram_tensor("posT", [3, CAND], f32r, kind="ExternalInput")
    wq_d = nc.dram_tensor("Wq2", [FEAT, FEAT], f32r, kind="ExternalInput")
    wnd_d = nc.dram_tensor("Wnd", [FEAT + 3, FEAT], f32r, kind="ExternalInput")
    id_d = nc.dram_tensor("I128", [128, 128], f32r, kind="ExternalInput")
    at_d = nc.dram_tensor("Atab", [128, QTILES * FEAT], f32r, kind="ExternalInput")
    neg_d = nc.dram_tensor("negones", [FEAT, 1], f32r, kind="ExternalInput")
    out_d = nc.dram_tensor("out", [HWP, FEAT], f32, kind="ExternalOutput")

    with tile.TileContext(nc) as tc:
        with (
            tc.tile_pool(name="const", bufs=1) as cpool,
            tc.tile_pool(name="zpsum", bufs=2, space="PSUM") as zp_pool,
            tc.tile_pool(name="apsum", bufs=1, space="PSUM") as ap_pool,
            tc.tile_pool(name="ypsum", bufs=1, space="PSUM") as yp_pool,
            tc.tile_pool(name="zsb", bufs=3) as zsb_pool,
            tc.tile_pool(name="small", bufs=4) as small_pool,
            tc.tile_pool(name="dram", bufs=1, space="DRAM") as dram_pool,
            tc.tile_pool(name="dram2", bufs=2, space="DRAM") as dram2_pool,
        ):
            # ---- constant loads ----
            ct = cpool.tile([KAUG, CAND], f32r)  # [c | pos^T | -|c|^2]
            for h in range(2):
                nc.sync.dma_start(
                    out=ct[0:FEAT, h * 1536 : (h + 1) * 1536],
                    in_=cT_d.ap()[:, h * 1536 : (h + 1) * 1536],
                )
            nc.sync.dma_start(out=ct[FEAT : FEAT + 3, :], in_=posT_d.ap())
            qt_f = cpool.tile([KAUG, HWP], f32)
            nc.sync.dma_start(out=qt_f[:], in_=qT_d.ap())
            wq = cpool.tile([FEAT, FEAT], f32r)
            nc.sync.dma_start(out=wq[:], in_=wq_d.ap())
            wnd = cpool.tile([FEAT + 3, FEAT], f32r)
            nc.sync.dma_start(out=wnd[:], in_=wnd_d.ap())
            ident = cpool.tile([128, 128], f32r)
            nc.sync.dma_start(out=ident[:], in_=id_d.ap())
            atab = cpool.tile([128, QTILES * FEAT], f32r)
            nc.sync.dma_start(out=atab[:], in_=at_d.ap())
            negones = cpool.tile([FEAT, 1], f32r)
            nc.sync.dma_start(out=negones[:], in_=neg_d.ap())

            # ---- query matrix [2q | 0 0 0 | 1] (host pads rows 64:67=0,
            # row 67=0.5; doubling gives the 1) ----
            qt = cpool.tile([KAUG, HWP], f32r)
            nc.scalar.mul(qt[:], qt_f[:], 2.0)

            # ---- -|c|^2 -> SBUF row (PSUM base 0, fp32r) -> ct row 67 ----
            sqt = cpool.tile([FEAT, CAND], f32r)
            sqrow = cpool.tile([1, CAND], f32r)
            for h in range(2):
                nc.scalar.square(
                    sqt[:, h * 1536 : (h + 1) * 1536],
                    ct[0:FEAT, h * 1536 : (h + 1) * 1536],
                )
                zp = zp_pool.tile([128, 1536], f32, tag="z")
                for s in range(3):
                    nc.tensor.matmul(
                        out=zp[0:1, s * 512 : (s + 1) * 512],
                        lhsT=negones[:],
                        rhs=sqt[:, h * 1536 + s * 512 : h * 1536 + (s + 1) * 512],
                        start=True,
                        stop=True,
                    )
                nc.scalar.copy(
                    out=sqrow[:, h * 1536 : (h + 1) * 1536], in_=zp[0:1, :]
                )
            nc.sync.dma_start(out=ct[FEAT + 3 : FEAT + 4, :], in_=sqrow[:])

            # ---- candidate table YP = [c|pos].Wnd -> DRAM (4-chunk groups) --
            ypd = dram_pool.tile([CAND, FEAT], f32)
            yp_sb = cpool.tile([128, CTILES * FEAT], f32)
            for grp in range(CTILES // 4):
                yp4 = yp_pool.tile([128, 4 * FEAT], f32, tag="yp")
                for j in range(4):
                    r = grp * 4 + j
                    nc.tensor.matmul(
                        out=yp4[:, j * FEAT : (j + 1) * FEAT],
                        lhsT=ct[0 : FEAT + 3, r * 128 : (r + 1) * 128],
                        rhs=wnd[:],
                        start=True,
                        stop=True,
                    )
                nc.scalar.copy(
                    out=yp_sb[:, grp * 4 * FEAT : (grp + 1) * 4 * FEAT], in_=yp4[:]
                )
            nc.sync.dma_start(
                out=ypd[:].rearrange("(g p) f -> p g f", p=128),
                in_=yp_sb[:].rearrange("p (g f) -> p g f", g=CTILES),
            )

            # ---- A bank: A = 2q.(Wq/2) + Atab, one PSUM bank, all tiles ----
            abank = ap_pool.tile([128, QTILES * FEAT], f32, tag="a")
            for q in range(QTILES):
                csl = slice(q * FEAT, (q + 1) * FEAT)
                nc.tensor.matmul(
                    out=abank[:, csl],
                    lhsT=qt[0:FEAT, q * 128 : (q + 1) * 128],
                    rhs=wq[:],
                    start=True,
                    stop=False,
                )
                nc.tensor.matmul(
                    out=abank[:, csl],
                    lhsT=ident[:],
                    rhs=atab[:, csl],
                    start=False,
                    stop=True,
                )

            # ---- per query tile ----
            for q in range(QTILES):
                qsl = slice(q * 128, (q + 1) * 128)
                zsb = zsb_pool.tile([128, CAND], f32, tag="zsb")
                for h in range(2):
                    zp = zp_pool.tile([128, 1536], f32, tag="z")
                    for s in range(3):
                        nc.tensor.matmul(
                            out=zp[:, s * 512 : (s + 1) * 512],
                            lhsT=qt[:, qsl],
                            rhs=ct[:, h * 1536 + s * 512 : h * 1536 + (s + 1) * 512],
                            start=True,
                            stop=True,
                        )
                    nc.scalar.copy(out=zsb[:, h * 1536 : (h + 1) * 1536], in_=zp[:])

                vals = small_pool.tile([128, K], f32, tag="vals")
                idx = small_pool.tile([128, K], mybir.dt.uint16, tag="idx")
                nc.vector.max(out=vals[:], in_=zsb[:])
                nc.vector.max_index(out=idx[:], in_max=vals[:], in_values=zsb[:])

                # shuffle into dma_gather's wrap layout (via DRAM bounce):
                # idxs_g[c, k*8+phi] = idx[phi*16+c, k], replicated per stripe
                d3 = dram2_pool.tile([128, K], i16, tag="d3")
                nc.scalar.dma_start(out=d3[:], in_=idx[:].bitcast(i16))
                idxs_g = small_pool.tile([128, 64], i16, tag="idxs_g")
                for k in range(K):
                    src = d3[:, k : k + 1].rearrange("(phi c) one -> c phi one", c=16)
                    nc.sync.dma_start(out=idxs_g[0:16, k * 8 : (k + 1) * 8], in_=src)
                nc.sync.dma_start(out=idxs_g[16:32, :], in_=idxs_g[0:16, :])
                nc.sync.dma_start(out=idxs_g[32:64, :], in_=idxs_g[0:32, :])
                nc.sync.dma_start(out=idxs_g[64:128, :], in_=idxs_g[0:64, :])

                g = small_pool.tile([128, K, FEAT], f32, tag="g")
                nc.gpsimd.dma_gather(
                    out_ap=g[:],
                    in_ap=ypd[:],
                    idxs_ap=idxs_g[:],
                    num_idxs=128 * K,
                    num_idxs_reg=128 * K,
                    elem_size=FEAT,
                    queue_num=q % 4,
                )

                gmax = small_pool.tile([128, FEAT], f32, tag="gmax")
                nc.vector.tensor_reduce(
                    out=gmax[:],
                    in_=g[:].rearrange("p k f -> p f k"),
                    op=mybir.AluOpType.max,
                    axis=mybir.AxisListType.X,
                )
                outsb = small_pool.tile([128, FEAT], f32, tag="outsb")
                nc.vector.tensor_add(
                    out=outsb[:], in0=gmax[:], in1=abank[:, q * FEAT : (q + 1) * FEAT]
                )
                nc.scalar.dma_start(out=out_d.ap()[qsl, :], in_=outsb[:])

    nc.compile()
    return nc


def _prep_in_maps(inputs):
    x = np.ascontiguousarray(np.asarray(inputs["x"], np.float32))
    W1 = np.asarray(inputs["W1"], np.float64)
    b1 = np.asarray(inputs["b1"], np.float64)
    W2 = np.asarray(inputs["W2"], np.float64)
    b2 = np.asarray(inputs["b2"], np.float64)
    W3 = np.asarray(inputs["W3"], np.float64)
    b3 = np.asarray(inputs["b3"], np.float64)

    Wc = W1.T @ W2.T @ W3.T                      # [131, 64]
    bc = b1 @ W2.T @ W3.T + b2 @ W3.T + b3       # [64]
    Wq2 = np.ascontiguousarray(Wc[:FEAT] / 2.0).astype(np.float32)
    Wn = Wc[FEAT : 2 * FEAT]
    Wd = Wc[2 * FEAT :]                          # [3, 64]
    Wnd = np.ascontiguousarray(np.vstack([Wn, Wd])).astype(np.float32)

    I128 = np.eye(128, dtype=np.float32)

    in_maps = []
    for c in range(NCORES):
        b, f = c // 4, c % 4
        frames = [t for t in range(T) if t != f]
        qT = np.zeros((KAUG, HWP), np.float32)
        qT[0:FEAT] = x[b, f].reshape(FEAT, HWP)
        qT[FEAT + 3] = 0.5
        cT = np.concatenate([x[b, t].reshape(FEAT, HWP) for t in frames], axis=1)

        jglob = np.concatenate(
            [np.arange(t * HWP, (t + 1) * HWP) for t in frames]
        )
        ctp = (jglob // HWP).astype(np.float64) / T
        chp = ((jglob % HWP) // W).astype(np.float64)
        cwp = ((jglob % HWP) % W).astype(np.float64)
        posT = np.ascontiguousarray(
            np.stack([ctp, chp, cwp], 0).astype(np.float32)
        )  # [3, 3072]

        iq = np.arange(f * HWP, (f + 1) * HWP)
        it = ((iq // H) * W).astype(np.float64) / T
        ih = (((iq % H) * W) // W).astype(np.float64)
        iw = (((iq % H) * W) % W).astype(np.float64)
        Atab = (bc + np.stack([it, ih, iw], -1) @ Wd).astype(np.float32)  # [1024,64]
        Atab_l = np.ascontiguousarray(
            Atab.reshape(QTILES, 128, FEAT).transpose(1, 0, 2).reshape(128, -1)
        )

        in_maps.append(
            {
                "qT": np.ascontiguousarray(qT),
                "cT": np.ascontiguousarray(cT),
                "posT": posT,
                "Wq2": Wq2,
                "Wnd": Wnd,
                "I128": I128,
                "negones": np.full((FEAT, 1), -1.0, np.float32),
                "Atab": Atab_l,
            }
        )
    return in_maps


def run_with_results(inputs, trace=False, **spmd_kwargs):
    """Run the SPMD kernel; returns (full_output, BassKernelResults)."""
    from concourse import bass_utils

    if "nc" not in _COMPILED:
        _COMPILED["nc"] = _build_nc()
    nc = _COMPILED["nc"]

    in_maps = _prep_in_maps(inputs)
    res = bass_utils.run_bass_kernel_spmd(
        nc, in_maps, core_ids=list(range(NCORES)), trace=trace, **spmd_kwargs
    )

    y = np.zeros((BS, THW, FEAT), np.float32)
    for c in range(NCORES):
        b, f = c // 4, c % 4
        y[b, f * HWP : (f + 1) * HWP] = res.results[c]["out"]
    out = y.reshape(BS, T, H, W, FEAT).transpose(0, 1, 4, 2, 3)
    return np.ascontiguousarray(out), res


def kernel(**inputs):
    out, _ = run_with_results(inputs, trace=False)
    return out
